# revision 1
# baseline (speedup 1.0000x reference)
"""Trainium2 Bass kernel for the rank-weighted log-loss reduction.

loss = -sum_i ri * (log(p_win_i) - R*(f0_i - P1)^2),  ri = i / (n*(n+1)/2)

Strategy (pure data parallel over 8 cores):
  - core k gets rows [k*M, (k+1)*M), M = N/8
  - on-chip: p_win via predicated copy, Ln + Square on the scalar engine,
    subtract into bf16, then a [128,3] x [128,512] matmul per chunk
    accumulates (sum per, sum pos_lo*per, sum pos_hi*per) per column into
    PSUM across all tiles. pos = 128*t + p is the row-chunk index; its
    lo/hi byte split keeps every weight exact in bf16.
  - host folds the per-core [3, F] partials into the closed-form weighted
    sum (weights are affine in (pos, f)) in float64.
"""

import numpy as np
import ml_dtypes
from contextlib import ExitStack

import concourse.bass as bass
import concourse.mybir as mybir
import concourse.tile as tile
from concourse.vector_clock import ScopedClock
from concourse.bass_utils import run_bass_kernel_spmd


MAX_SYNC_WAITS = 1


def _spill_excess_waits(nc, max_waits=MAX_SYNC_WAITS):
    """The walrus in this toolchain rejects instructions carrying more than
    a couple of sync waits ("Too many sync wait commands"). Spill the excess
    onto same-engine NOPs inserted immediately before — semantically
    identical (consecutive sem-ge waits on one engine)."""
    import bass_rust

    k = 0
    for f in nc.m.functions:
        for b in f.blocks:
            out = []
            changed = False
            for inst in b.instructions:
                si = inst.sync_info
                waits = list(si.on_wait or []) if si is not None else []
                if len(waits) > max_waits:
                    chunks = [
                        waits[i : i + max_waits]
                        for i in range(0, len(waits), max_waits)
                    ]
                    for chunk in chunks[:-1]:
                        nop = mybir.InstNoOp(name=f"antspill-{k}", ins=[], outs=[])
                        k += 1
                        nop.engine = inst.engine
                        nop.sync_info = bass_rust.SyncInfo(
                            on_wait=chunk, on_update=[]
                        )
                        out.append(nop)
                    inst.sync_info = bass_rust.SyncInfo(
                        on_wait=chunks[-1], on_update=list(si.on_update or [])
                    )
                    changed = True
                out.append(inst)
            if changed:
                b.instructions = out

N_TOTAL = 16777216
N_CORES = 8
P = 128          # SBUF partitions
F = 1024         # rows per partition per tile
T = 16           # tiles per core; P*F*T = 2097152 = N_TOTAL/N_CORES
R = 1.0
P1 = 0.5


def build_nc(F=F, T=T):
    M = P * F * T
    nc = bass.Bass(
        "TRN2", target_bir_lowering=False, debug=False,
        enable_asserts=False, num_devices=1,
    )
    fo = nc.dram_tensor("fo", [M, 2], mybir.dt.float32, kind="ExternalInput")
    pv = nc.dram_tensor("pv", [M], mybir.dt.int32, kind="ExternalInput")
    wt = nc.dram_tensor("wt", [P, 3 * T], mybir.dt.bfloat16, kind="ExternalInput")
    out = nc.dram_tensor("out", [3, F], mybir.dt.float32, kind="ExternalOutput")

    fo_r = fo.ap().rearrange("(t p f) c -> t p f c", t=T, p=P, f=F)
    pv_r = pv.ap().rearrange("(t p f) -> t p f", t=T, p=P, f=F)

    with tile.TileContext(nc) as tc, ExitStack() as ctx:
        xp = ctx.enter_context(tc.tile_pool(name="xp", bufs=3))
        vp = ctx.enter_context(tc.tile_pool(name="vp", bufs=3))
        mp = ctx.enter_context(tc.tile_pool(name="mp", bufs=3))
        cp = ctx.enter_context(tc.tile_pool(name="cp", bufs=1))
        ps = ctx.enter_context(tc.tile_pool(name="ps", bufs=1, space="PSUM"))

        W = cp.tile([P, 3 * T], mybir.dt.bfloat16)
        nc.sync.dma_start(W[:], wt[:])
        nbias = cp.tile([P, 1], mybir.dt.float32)
        nc.vector.memset(nbias[:], -P1)
        acc = ps.tile([3, F], mybir.dt.float32)
        for t in range(T):
            X = xp.tile([P, F, 2], mybir.dt.float32, tag="X")
            V = vp.tile([P, F], mybir.dt.int32, tag="V")
            nc.sync.dma_start(X[:], fo_r[t])
            nc.sync.dma_start(V[:], pv_r[t])
            pw = mp.tile([P, F], mybir.dt.float32, tag="pw")
            nc.vector.tensor_copy(pw[:], X[:, :, 0])
            nc.vector.copy_predicated(pw[:], V[:], X[:, :, 1])
            lp = mp.tile([P, F], mybir.dt.float32, tag="lp")
            nc.scalar.activation(lp[:], pw[:], mybir.ActivationFunctionType.Ln)
            sq = mp.tile([P, F], mybir.dt.float32, tag="sq")
            nc.scalar.activation(
                sq[:], X[:, :, 0], mybir.ActivationFunctionType.Square, bias=nbias[:]
            )
            per = mp.tile([P, F], mybir.dt.bfloat16, tag="per")
            nc.vector.tensor_sub(per[:], lp[:], sq[:])
            for h0 in range(0, F, 512):
                sl = slice(h0, min(h0 + 512, F))
                nc.tensor.matmul(
                    acc[:, sl], W[:, 3 * t : 3 * (t + 1)], per[:, sl],
                    start=(t == 0), stop=(t == T - 1),
                )
        ob = cp.tile([3, F], mybir.dt.float32)
        nc.vector.tensor_copy(ob[:], acc[:])
        nc.sync.dma_start(out[:], ob[:])
    _spill_excess_waits(nc)
    return nc


def build_wt(T=T):
    """Per-tile stationary matrix: columns (ones, pos_lo, pos_hi), where
    pos = 128*t + p is the row-chunk index. lo/hi split keeps values exact
    in bf16 (lo < 256; hi a multiple of 256 <= 2^8*T)."""
    cols = np.zeros((P, 3 * T), np.float32)
    p_idx = np.arange(P, dtype=np.int64)
    for t in range(T):
        pos = t * P + p_idx
        lo = pos & 255
        hi = pos - lo
        cols[:, 3 * t] = 1.0
        cols[:, 3 * t + 1] = lo
        cols[:, 3 * t + 2] = hi
    return cols.astype(ml_dtypes.bfloat16)


def combine(outs, F=F, T=T):
    """Fold per-core [3, F] partials into the loss.

    Row i = k*M + pos*F + f. Per core:
      sum_i per_i * i = k*M*S + F*(sum pos*per) + (sum f*per)
    with S = sum(c0), sum pos*per = sum(c_lo + c_hi), sum f*per = sum(f*c0).
    """
    M = P * F * T
    n = M * len(outs)
    # mirror the reference's fp32 denom computation
    denom = float(np.float32(n) * np.float32(n + 1) * np.float32(0.5))
    j = np.arange(F, dtype=np.float64)
    total = 0.0
    for k, o in enumerate(outs):
        c0 = o[0].astype(np.float64)
        cw = o[1].astype(np.float64) + o[2].astype(np.float64)
        total += (k * M) * c0.sum() + F * cw.sum() + (j * c0).sum()
    return -total / denom


_NC_CACHE = {}


def _run(final_out, point_victor, **spmd_kwargs):
    fo = np.ascontiguousarray(np.asarray(final_out, dtype=np.float32))
    pv = np.ascontiguousarray(np.asarray(point_victor, dtype=np.int32))
    assert fo.shape == (N_TOTAL, 2) and pv.shape == (N_TOTAL,)
    M = N_TOTAL // N_CORES

    if "nc" not in _NC_CACHE:
        _NC_CACHE["nc"] = build_nc()
    nc = _NC_CACHE["nc"]
    wt = build_wt()

    in_maps = [
        {"fo": fo[k * M : (k + 1) * M], "pv": pv[k * M : (k + 1) * M], "wt": wt}
        for k in range(N_CORES)
    ]
    res = run_bass_kernel_spmd(nc, in_maps, core_ids=list(range(N_CORES)), **spmd_kwargs)
    outs = [r["out"] for r in res.results]
    return np.float32(combine(outs)), res


def kernel(final_out, point_victor):
    return _run(final_out, point_victor)[0]

